# revision 16
# baseline (speedup 1.0000x reference)
"""Trainium2 Bass kernel for nn_AttentionAggregator (gnn_message_passing).

reference:
    feats  = embed_table[neigh_idx]            # [N, K, D] gather
    scores = einsum("nkd,d->nk", feats, attn_w)
    wts    = softmax(scores, axis=1)
    out    = einsum("nk,nkd->nd", wts, feats)  # [N, D]

Strategy (8 cores, data-parallel over N):
  - Each core owns 12544 (padded) target nodes = 98 tiles of 128 nodes.
  - Per (tile, k): one indirect DMA gathers the k-th neighbor row of all
    128 nodes (one offset per partition -- the granularity this walrus
    build's indirect DMA supports on hardware).
  - Scores via fused scalar_tensor_tensor with accum_out (one DVE op per
    (tile,k): junk = F_k * w, accum_out = per-partition dot).
  - Per-tile softmax over K=10 in the free dim, normalization folded into
    the weights.
  - Weighted sum: acc = F_k * w_k + acc via scalar_tensor_tensor with a
    per-partition scalar weight.

Self-contained: hardcodes all shapes; imports only the system concourse
stack.
"""

import numpy as np

# ---------------------------------------------------------------- problem dims
N_CORES = 8
N = 100000          # target nodes (global)
K = 10              # neighbors per node
VOCAB = 200000      # embedding rows
D = 128             # embedding dim

P = 128             # SBUF partitions
NT = 98             # node tiles per core (128 nodes each)
NPC = NT * P        # padded nodes per core = 12544 (>= 100000/8 = 12500)

_CACHE = {}


# ------------------------------------------------------------------ walrus fix
def _install_tile_drain_patch():
    """This walrus build allows only ONE sync-wait per CTRL instruction; the
    TileContext tail drain can collect several.  Split them across preceding
    SP nops."""
    import bass_rust
    from concourse.tile import TileContext
    from concourse.vector_clock import ScopedClock

    if getattr(TileContext, "_drain_patch_installed", False):
        return

    def _drain_and_barrier(self, tick_clock, wait_clock):
        nop = self.nc.sync.nop()
        wait_clock.add_sem_waits(nop.ins, ScopedClock({None: tick_clock.global_clock}))
        si = nop.ins.sync_info
        waits = list(si.on_wait) if si is not None else []
        if len(waits) > 1:
            nop.ins.sync_info = bass_rust.SyncInfo(
                on_wait=waits[:1], on_update=list(si.on_update)
            )
            for i in range(1, len(waits)):
                extra = self.nc.sync.nop()
                extra.ins.sync_info = bass_rust.SyncInfo(
                    on_wait=waits[i : i + 1], on_update=[]
                )
        self.nc.sync.drain()
        self.nc.all_engine_barrier()
        assert self.sems is not None
        popped = self.nc._tile_sem_poison_stack.pop()
        assert popped is self._sem_poison
        self.nc.clear_and_free_semaphores(list(self.sems.allocated().values()))
        self.nc.all_engine_barrier()

    TileContext._drain_and_barrier = _drain_and_barrier
    TileContext._drain_patch_installed = True


def _split_multi_waits(nc):
    """Post-pass: this walrus build rejects >1 sync-wait per instruction.
    For any instruction carrying n>1 waits, insert n-1 single-wait nops on
    the same engine immediately before it (engine program order is the bb
    order filtered by engine, so all waits still complete before it runs)."""
    import bass_rust
    import concourse.mybir as mybir

    n_split = 0
    for f in nc.m.functions:
        for bb in f.blocks:
            new_list = []
            changed = False
            for inst in bb.instructions:
                si = inst.sync_info
                if si is not None and len(si.on_wait) > 1:
                    waits = list(si.on_wait)
                    for w in waits[:-1]:
                        nop = mybir.InstNoOp(
                            name=f"{inst.name}_wsplit{n_split}",
                            engine=inst.engine,
                            bass_nofuse=True,
                            sync_info=bass_rust.SyncInfo(
                                on_wait=[w], on_update=[]
                            ),
                        )
                        n_split += 1
                        nc.register_instruction(nop, overwrite=True)
                        new_list.append(nop)
                    inst.sync_info = bass_rust.SyncInfo(
                        on_wait=[waits[-1]], on_update=list(si.on_update)
                    )
                    changed = True
                new_list.append(inst)
            if changed:
                bb.instructions = new_list
    return n_split


# ------------------------------------------------------------- device program
def build_program(repeats=1):
    """Per-core Bass program.  repeats>1 python-unrolls the computation for
    wall-clock timing (dispatch overhead cancels in the difference)."""
    _install_tile_drain_patch()
    from contextlib import ExitStack

    import concourse.bass as bass
    import concourse.mybir as mybir
    from concourse.bass import IndirectOffsetOnAxis
    from concourse.tile import TileContext

    f32 = mybir.dt.float32
    i32 = mybir.dt.int32
    Alu = mybir.AluOpType

    nc = bass.Bass("TRN2", target_bir_lowering=False, debug=False,
                   num_devices=N_CORES)

    table = nc.dram_tensor("embed_table", [VOCAB, D], f32,
                           kind="ExternalInput").ap()
    idx = nc.dram_tensor("neigh_idx", [NPC, K], i32, kind="ExternalInput").ap()
    attn = nc.dram_tensor("attn_w", [P, D], f32, kind="ExternalInput").ap()
    out = nc.dram_tensor("out", [NPC, D], f32, kind="ExternalOutput").ap()

    with TileContext(nc) as tc, ExitStack() as ctx:
        const = ctx.enter_context(tc.tile_pool(name="const", bufs=1))
        w_sb = const.tile([P, D], f32)
        nc.sync.dma_start(out=w_sb[:], in_=attn[:])
        idx_sb = const.tile([P, NT * K], i32)   # node layout: [p, t*K+k]
        junk = const.tile([P, D], f32)

        fp = ctx.enter_context(tc.tile_pool(name="feats", bufs=3))
        sp = ctx.enter_context(tc.tile_pool(name="smax", bufs=4))
        op_ = ctx.enter_context(tc.tile_pool(name="acc", bufs=4))

        out_t = out.rearrange("(t p) d -> t p d", p=P)

        for _rep in range(repeats):
            nc.sync.dma_start(
                out=idx_sb.rearrange("p (t k) -> p t k", k=K),
                in_=idx.rearrange("(t p) k -> p t k", p=P),
            )
            for t in range(NT):
                fb = fp.tile([P, K * D], f32)       # 10 neighbor rows/node
                fb3 = fb.rearrange("p (k d) -> p k d", d=D)
                sc = sp.tile([P, K], f32)           # scores -> weights
                for k in range(K):
                    nc.gpsimd.indirect_dma_start(
                        out=fb3[:, k],
                        out_offset=None,
                        in_=table,
                        in_offset=IndirectOffsetOnAxis(
                            ap=idx_sb[:, t * K + k : t * K + k + 1], axis=0
                        ),
                    )
                    nc.vector.scalar_tensor_tensor(
                        out=junk[:], in0=fb3[:, k], scalar=1.0, in1=w_sb[:],
                        op0=Alu.mult, op1=Alu.mult,
                        accum_out=sc[:, k : k + 1],
                    )
                # softmax over k (free dim), normalization folded in
                mx = sp.tile([P, 1], f32, tag="mx")
                sm = sp.tile([P, 1], f32, tag="sm")
                nc.vector.tensor_reduce(
                    out=mx[:], in_=sc[:], axis=mybir.AxisListType.X, op=Alu.max
                )
                nc.vector.tensor_scalar(
                    sc[:], sc[:], mx[:, 0:1], None, Alu.subtract
                )
                nc.scalar.activation(
                    out=sc[:], in_=sc[:], func=mybir.ActivationFunctionType.Exp
                )
                nc.vector.tensor_reduce(
                    out=sm[:], in_=sc[:], axis=mybir.AxisListType.X, op=Alu.add
                )
                nc.vector.reciprocal(out=sm[:], in_=sm[:])
                nc.vector.tensor_scalar(
                    sc[:], sc[:], sm[:, 0:1], None, Alu.mult
                )
                # weighted sum: acc = sum_k w_k * F_k
                acc = op_.tile([P, D], f32)
                nc.vector.tensor_scalar(
                    acc[:], fb3[:, 0], sc[:, 0:1], None, Alu.mult
                )
                for k in range(1, K):
                    nc.vector.scalar_tensor_tensor(
                        out=acc[:],
                        in0=fb3[:, k],
                        scalar=sc[:, k : k + 1],
                        in1=acc[:],
                        op0=Alu.mult,
                        op1=Alu.add,
                    )
                nc.sync.dma_start(out=out_t[t], in_=acc[:])

    _split_multi_waits(nc)
    return nc


# ------------------------------------------------------------------ host side
def _prep_inputs(embed_table, attn_w, neigh_idx):
    embed_table = np.ascontiguousarray(np.asarray(embed_table, dtype=np.float32))
    attn_w = np.ascontiguousarray(
        np.broadcast_to(np.asarray(attn_w, dtype=np.float32).reshape(1, D), (P, D))
    )
    idx = np.asarray(neigh_idx)
    assert idx.shape == (N, K)
    idx32 = idx.astype(np.int32)
    idx_pad = np.zeros((N_CORES * NPC, K), dtype=np.int32)
    idx_pad[:N] = idx32
    in_maps = []
    for c in range(N_CORES):
        in_maps.append(
            {
                "embed_table": embed_table,
                "neigh_idx": idx_pad[c * NPC : (c + 1) * NPC],
                "attn_w": attn_w,
            }
        )
    return in_maps


class _Runner:
    """jit-once SPMD runner on the axon/PJRT path (device-resident inputs)."""

    def __init__(self, nc):
        import jax
        from jax.experimental.shard_map import shard_map
        from jax.sharding import Mesh, NamedSharding, PartitionSpec

        import concourse.mybir as mybir
        from concourse.bass2jax import (
            _bass_exec_p,
            install_neuronx_cc_hook,
            partition_id_tensor,
        )

        install_neuronx_cc_hook()
        self.jax = jax
        partition_name = (
            nc.partition_id_tensor.name if nc.partition_id_tensor else None
        )

        in_names, out_names, out_avals, zero_outs = [], [], [], []
        for alloc in nc.m.functions[0].allocations:
            if not isinstance(alloc, mybir.MemoryLocationSet):
                continue
            name = alloc.memorylocations[0].name
            if alloc.kind == "ExternalInput":
                if name != partition_name:
                    in_names.append(name)
            elif alloc.kind == "ExternalOutput":
                out_names.append(name)
                shape = tuple(alloc.tensor_shape)
                dtype = mybir.dt.np(alloc.dtype)
                out_avals.append(jax.core.ShapedArray(shape, dtype))
                zero_outs.append(np.zeros(shape, dtype))
        self.in_names = in_names
        self.out_names = out_names
        self.zero_outs = zero_outs
        n_params = len(in_names)
        n_outs = len(out_avals)
        all_in = list(in_names) + list(out_names)
        if partition_name is not None:
            all_in.append(partition_name)

        def _body(*args):
            operands = list(args)
            if partition_name is not None:
                operands.append(partition_id_tensor())
            outs = _bass_exec_p.bind(
                *operands,
                out_avals=tuple(out_avals),
                in_names=tuple(all_in),
                out_names=tuple(out_names),
                lowering_input_output_aliases=(),
                sim_require_finite=True,
                sim_require_nnan=True,
                nc=nc,
            )
            return tuple(outs)

        devices = jax.devices()[:N_CORES]
        assert len(devices) == N_CORES
        mesh = Mesh(np.asarray(devices), ("core",))
        self.mesh = mesh
        self.sharding = NamedSharding(mesh, PartitionSpec("core"))
        self.fn = jax.jit(
            shard_map(
                _body,
                mesh=mesh,
                in_specs=(PartitionSpec("core"),) * (n_params + n_outs),
                out_specs=(PartitionSpec("core"),) * n_outs,
                check_rep=False,
            ),
            keep_unused=True,
        )

    def put_inputs(self, in_maps):
        args = []
        for name in self.in_names:
            glob = np.concatenate(
                [np.asarray(m[name]) for m in in_maps], axis=0
            )
            args.append(self.jax.device_put(glob, self.sharding))
        for z in self.zero_outs:
            glob = np.concatenate([z] * N_CORES, axis=0)
            args.append(self.jax.device_put(glob, self.sharding))
        return args

    def run(self, args):
        outs = self.fn(*args)
        self.jax.block_until_ready(outs)
        return outs


def get_runner():
    if "runner" not in _CACHE:
        nc = build_program()
        _CACHE["runner"] = _Runner(nc)
    return _CACHE["runner"]


def kernel(embed_table, attn_w, neigh_idx):
    """Full-input, full-output entry point (grading contract)."""
    r = get_runner()
    in_maps = _prep_inputs(embed_table, attn_w, neigh_idx)
    args = r.put_inputs(in_maps)
    outs = r.run(args)
    glob = np.asarray(outs[r.out_names.index("out")])  # [8*NPC, D]
    # cores hold contiguous padded slices; padding sits at the global tail
    return glob[:N].astype(np.float32)
